# revision 36
# baseline (speedup 1.0000x reference)
"""Sparse-attention distance-mask kernel for Trainium2 (8 NeuronCores).

Reference computation (per batch b):
    pos      = multi-hot of 4 tree-position ids over 512 nodes   [seq, 512]
    dist     = s_i + s_j - 2 * pos @ pos.T          (L1 dist of binary vecs)
    attn     = max(dist_top, dist_left)
    out      = attn + padding_dist * max(pad_i, pad_j)

Kernel strategy (one batch per core; b == n_cores == 8):
  - +/-1 encoding: with q = 1 - 2*pos, dist = 256 - <q_i,q_j>/2 — the
    s_i/s_j rank terms vanish and lhsT == rhs == q, so each mask needs
    ONE fp8 tensor (inputs total ~1.1 MB; the kernel is DMA-bound).
  - fp8 DoubleRow matmuls: K=512 in 2 passes per mask per block,
    ~216 ns per 512-wide pass warm (2x over plain fp8).
  - A symmetric permutation sorts PAD positions last per batch.  The
    padding matrix A = 256 + p*max(pad_i,pad_j) decomposes into a
    per-partition row bias bp_i = 256 + p*pad_i plus a column term
    p*pad_j*(1-pad_i) confined to the last fw output columns (one tiny
    gated op per band).
  - Per block only 2 engine ops, via a custom DVE op AFFINE_THEN_MAX
    (out = (in0*s0 + s1) max in1) registered at build time:
      x  = ACT Identity(ps_top * -0.5 + bp_i)          (scalar engine)
      cp = DVE (ps_left * -0.5 + bp_i) max x           (custom, 1 op)
  - cp / the output DRAM tensor are bf16: every output value is an
    integer <= ~460, exactly representable — stores halve to 1.2 MB
    with zero error; host converts back to f32.
  - Only the upper block-triangle (128-row granularity) is computed;
    the rest is mirrored on host, then rows/cols are inverse-permuted.
  - Warm-up matmuls on scratch release the PE clock gate during the
    input DMA fill; DMA triggers (~650ns each) are spread across the
    SP and ACT queues.
"""

import os

import ml_dtypes
import numpy as np

B, SEQ, DEPTH = 8, 1024, 4
TN = 512          # TOTAL_NODE
N_CORES = 8
MB = SEQ // 128
N_WARMUP = 6
PIPE_LAG = 2

# per 128-row band: col blocks (col0, width), first block narrow, then 512s
ROW_BLOCKS = {}
for mb in range(MB):
    lo = mb * 128
    blocks = []
    rem = (SEQ - lo) % 512
    if rem:
        blocks.append((lo, rem))
        lo += rem
    while lo < SEQ:
        blocks.append((lo, 512))
        lo += 512
    ROW_BLOCKS[mb] = blocks

_NC_CACHE = {}
_ATM_OP = None
LAST_RESULTS = None


def _register_affine_then_max():
    """Register the custom DVE op  out = (in0*s0 + s1) max in1."""
    global _ATM_OP
    if _ATM_OP is not None:
        return _ATM_OP
    from concourse import dve_ops as dops
    from concourse.dve_spec import C0, C1, Spec, Src0, Src1, maxx

    name = "AFFINE_THEN_MAX"
    existing = [op for op in dops.OPS if op.name == name]
    if existing:
        _ATM_OP = existing[0]
        return _ATM_OP

    spec = Spec(
        body=maxx(Src0 * C0 + C1, Src1),
        reference=lambda in0, in1, s0, s1, imm2: np.maximum(
            in0.astype(np.float32) * s0 + s1, in1
        ),
    )
    op = dops.DveOp(name, spec, subdim=False, uops_sha={})
    dops.OPS.append(op)
    dops.CUSTOM_DVE_SPECS[name] = spec
    dops._SUB_OPCODE_FOR_NAME[name] = max(
        dops._SUB_OPCODE_FOR_NAME.values()) + 1
    # pin the uops sha (compile() raises with the actual value)
    for ver in ("v3", "v4"):
        try:
            op.compile(ver)
        except ValueError as e:
            msg = str(e)
            got = msg.split(f"{ver}: ")[1].split(" ")[0]
            object.__setattr__(op, "uops_sha", {**op.uops_sha, ver: got})
    _ATM_OP = op
    return op


def _aug_factor(p):
    """Find c1*c2 == p with c1, c2 fp8(e4m3)-exact; None if impossible."""
    def _exact(x):
        return float(np.float32(x).astype(ml_dtypes.float8_e4m3)) == float(
            np.float32(x))
    for k in range(-6, 8):
        for m in range(8):
            c2 = float(np.float32(2.0 ** k) * np.float32(1 + m / 8.0))
            if c2 == 0:
                continue
            c1 = float(np.float32(p) / np.float32(c2))
            if float(np.float32(c1) * np.float32(c2)) == float(
                    np.float32(p)) and _exact(c1) and _exact(c2):
                return c1, c2
    return None


def _build_nc(fw, fused_fix):
    """fw: width of the pad-column fix region (multiple of 128, >= 128).
    fused_fix: add the pad-col term via K=1 aug matmuls into both PSUMs
    (needs p fp8-factorable); else one DVE fix op per band."""
    import concourse.mybir as mybir
    from concourse import bacc
    from concourse.tile import TileContext

    atm = _register_affine_then_max()

    DR = mybir.MatmulPerfMode.DoubleRow
    ALU = mybir.AluOpType
    ACTF = mybir.ActivationFunctionType
    CFIX = SEQ - fw

    nc = bacc.Bacc()
    dram = {}
    for name in ("qt", "ql"):
        # chunk-major: two contiguous k-tile-pair halves
        dram[name] = nc.dram_tensor(
            name, [2, 128, 2, SEQ], mybir.dt.float8e4, kind="ExternalInput"
        )
    # aux packs bp | G | Vp into one load: [128, 8 + 8 + fw]
    dram["aux"] = nc.dram_tensor("aux", [128, 2 * MB + fw], mybir.dt.float32,
                                 kind="ExternalInput")
    if fused_fix:
        # ag row 0: [0:SEQ] = -2*c1*(1-pad_i); [SEQ:] = c2*pad_j.
        # Rows 1..31 are zeros: K<32 matmuls read the full 32-row tile, so
        # the unused rows must hold defined zeros, not neighbouring tiles.
        dram["ag"] = nc.dram_tensor("ag", [32, 2, SEQ + fw],
                                    mybir.dt.float8e4, kind="ExternalInput")
    out = nc.dram_tensor("out", [SEQ, SEQ], mybir.dt.bfloat16,
                         kind="ExternalOutput")
    debug = os.environ.get("KDEBUG", "") == "1"
    if debug:
        dram["dbg_ag"] = nc.dram_tensor("dbg_ag", [32, 2 * (SEQ + fw)],
                                        mybir.dt.float8e4,
                                        kind="ExternalOutput")
        dram["dbg_x0"] = nc.dram_tensor("dbg_x0", [128, SEQ],
                                        mybir.dt.float32,
                                        kind="ExternalOutput")

    with TileContext(nc) as tc:
        with (
            tc.tile_pool(name="w", bufs=1) as wpool,
            tc.tile_pool(name="pst", bufs=3, space="PSUM") as tpool,
            tc.tile_pool(name="psl", bufs=3, space="PSUM") as lpool,
            tc.tile_pool(name="psw", bufs=1, space="PSUM") as wmpool,
            tc.tile_pool(name="ep", bufs=1) as epool,
        ):
            q = {}
            for name in ("qt", "ql"):
                q[name] = wpool.tile([128, 4, SEQ], mybir.dt.float8e4,
                                     tag=name, name=name)
            aux = wpool.tile([128, 2 * MB + fw], mybir.dt.float32, tag="aux",
                             name="aux")
            bp = aux[:, 0:MB]
            G = aux[:, MB:2 * MB]
            Vp = aux[:, 2 * MB:]
            actw = wpool.tile([128, 1], mybir.dt.float32, tag="actw",
                              name="actw")
            if fused_fix:
                ag = wpool.tile([32, 2, SEQ + fw], mybir.dt.float8e4,
                                tag="ag", name="ag")

            # PE warm-up on scratch: releases the HAM clock gate while the
            # input DMAs fill SBUF.  Results never read.
            scratch = wpool.tile([128, 2, 512], mybir.dt.float8e4,
                                 tag="scratch", name="scratch")
            nc.gpsimd.memset(scratch[:, :, :], 0.0)
            ps_w = wmpool.tile([128, 512], mybir.dt.float32, tag="pw",
                               name="ps_warm")
            for _ in range(N_WARMUP):
                nc.tensor.matmul(ps_w[:, :], lhsT=scratch[:, :, 0:128],
                                 rhs=scratch[:, :, :], start=True, stop=True,
                                 perf_mode=DR)

            # input loads: triggers cost ~650ns each on the issuing engine's
            # sequencer, so spread them across sync and scalar
            def chunk_load(eng, name, c):
                eng.dma_start(out=q[name][:, 2 * c:2 * c + 2, :],
                              in_=dram[name][c])

            chunk_load(nc.sync, "qt", 0)
            chunk_load(nc.scalar, "qt", 1)
            chunk_load(nc.sync, "ql", 0)
            chunk_load(nc.scalar, "ql", 1)
            nc.sync.dma_start(out=aux[:, :], in_=dram["aux"][:, :])
            if fused_fix:
                nc.scalar.dma_start(out=ag[:, :, :],
                                    in_=dram["ag"][:, :, :])

            # early tiny ACT op: pulls the Identity act-table load into the
            # DMA window instead of stalling the first real epilogue op
            nc.scalar.activation(actw[:, :], bp[:, 0:1], ACTF.Identity,
                                 bias=bp[:, 0:1], scale=1.0)

            # per-band output tiles sized to the computed col range (bf16 —
            # all output values are small integers, exactly representable)
            cps = {}
            xs = {}
            for mb in range(MB):
                wid = SEQ - mb * 128
                cps[mb] = epool.tile([128, wid], mybir.dt.bfloat16,
                                     tag=f"cp{mb}", name=f"cp{mb}")
                xs[mb] = epool.tile([128, wid], mybir.dt.bfloat16,
                                    tag=f"x{mb}", name=f"x{mb}")

            def tslice(tiles, mb, c0, w):
                off = c0 - mb * 128
                return tiles[mb][:, off:off + w]

            def gemm(psum, name, mb, c0, w):
                t = q[name]
                m0 = mb * 128
                aug = fused_fix and c0 + w == SEQ
                nc.tensor.matmul(psum[:, 0:w],
                                 lhsT=t[:, 0:2, m0:m0 + 128],
                                 rhs=t[:, 0:2, c0:c0 + w],
                                 start=True, stop=False, perf_mode=DR)
                nc.tensor.matmul(psum[:, 0:w],
                                 lhsT=t[:, 2:4, m0:m0 + 128],
                                 rhs=t[:, 2:4, c0:c0 + w],
                                 start=False, stop=not aug, perf_mode=DR)
                if aug:
                    # rank-1 pad-col term: -2 * (-c1/2*(1-pad_i)) * c2*pad_j
                    f0 = max(CFIX, mb * 128)
                    nc.tensor.matmul(
                        psum[:, f0 - c0:w],
                        lhsT=ag[0:32, 0:2, m0:m0 + 128],
                        rhs=ag[0:32, 0:2, SEQ + f0 - CFIX:],
                        start=False, stop=True, skip_group_check=True,
                        perf_mode=DR)

            ordered = [(mb, c0, w) for mb in range(MB)
                       for (c0, w) in ROW_BLOCKS[mb]]

            def blk_a(mb, c0, w):
                # top GEMM -> x = ACT(ps * -0.5 + bp_i) = dist_t + p*pad_i
                ps_t = tpool.tile([128, 512], mybir.dt.float32, tag="pt",
                                  name=f"pt{mb}_{c0}")
                gemm(ps_t, "qt", mb, c0, w)
                nc.scalar.activation(tslice(xs, mb, c0, w), ps_t[:, 0:w],
                                     ACTF.Identity, bias=bp[:, mb:mb + 1],
                                     scale=-0.5)

            def blk_b(mb, c0, w):
                # left GEMM -> cp = (ps * -0.5 + bp_i) max x; pad-col fix on
                # the band's last block, then store the band
                ps_l = lpool.tile([128, 512], mybir.dt.float32, tag="pl",
                                  name=f"pl{mb}_{c0}")
                gemm(ps_l, "ql", mb, c0, w)
                sl = tslice(cps, mb, c0, w)
                nc.vector._custom_dve(
                    atm, out=sl, in0=ps_l[:, 0:w],
                    in1=tslice(xs, mb, c0, w), s0=-0.5, s1=bp[:, mb:mb + 1])
                if c0 + w == SEQ:
                    if not fused_fix:
                        # cp[:, CFIX:] += Vp * (1 - pad_i)
                        f0 = max(CFIX, mb * 128)
                        fsl = tslice(cps, mb, f0, SEQ - f0)
                        nc.vector.scalar_tensor_tensor(
                            out=fsl, in0=Vp[:, f0 - CFIX:],
                            scalar=G[:, mb:mb + 1], in1=fsl,
                            op0=ALU.mult, op1=ALU.add)
                    # whole band done: store it (sync queue — the scalar
                    # engine is busy with op1s and would delay the trigger)
                    ms = slice(mb * 128, (mb + 1) * 128)
                    nc.sync.dma_start(out=out[ms, mb * 128:], in_=cps[mb][:, :])

            # software-pipeline: B-blocks trail A-blocks by PIPE_LAG so the
            # epilogue overlaps the GEMM stream while the first A-blocks only
            # need the top tensor (loaded first)
            for i, (mb, c0, w) in enumerate(ordered):
                blk_a(mb, c0, w)
                if i >= PIPE_LAG:
                    blk_b(*ordered[i - PIPE_LAG])
            for j in range(max(0, len(ordered) - PIPE_LAG), len(ordered)):
                blk_b(*ordered[j])
            if debug:
                if fused_fix:
                    nc.sync.dma_start(out=dram["dbg_ag"][:, :],
                                      in_=ag[:, :, :])
                nc.sync.dma_start(out=dram["dbg_x0"][:, :], in_=xs[0][:, :])
    nc.compile()
    return nc


def _host_prep(zipped_top, zipped_left, indicator, p):
    """Permute pads last, build fp8 operands and epilogue tensors."""
    fp8 = ml_dtypes.float8_e4m3
    pad = (np.asarray(indicator) == 0)
    b, seq = pad.shape
    # stable sort: non-pads first, pads last
    perms = np.argsort(pad, axis=1, kind="stable")
    pad_p = np.take_along_axis(pad, perms, axis=1).astype(np.float32)

    npad_max = int(pad.sum(axis=1).max())
    fw = max(128, 128 * -(-npad_max // 128))

    ins = {}
    for key, zipped in (("qt", zipped_top), ("ql", zipped_left)):
        z = np.asarray(zipped, dtype=np.int64)
        z = np.take_along_axis(z, perms[:, :, None], axis=1)
        oh = np.zeros((b, seq, TN + 1), dtype=np.float32)
        np.put_along_axis(oh, z, 1.0, axis=2)
        qv = 1.0 - 2.0 * oh[..., :TN]                  # [b, seq, 512] +/-1
        kt = qv.transpose(0, 2, 1).reshape(b, 2, 2, 128, seq)
        ins[key] = np.ascontiguousarray(
            kt.transpose(0, 1, 3, 2, 4)).astype(fp8)

    pad_b = pad_p.reshape(b, MB, 128).transpose(0, 2, 1)   # [b,128,MB]
    aux = np.empty((b, 128, 2 * MB + fw), dtype=np.float32)
    aux[:, :, 0:MB] = 256.0 + p * pad_b
    aux[:, :, MB:2 * MB] = 1.0 - pad_b
    aux[:, :, 2 * MB:] = p * pad_p[:, None, SEQ - fw:]
    ins["aux"] = aux

    fac = None  # fused aug measured slower (deferred groups stall PE)
    fused_fix = fac is not None
    if fused_fix:
        c1, c2 = fac
        ag = np.zeros((b, 32, 2, SEQ + fw), dtype=np.float32)
        ag[:, 0, 0, 0:SEQ] = -c1 * (1.0 - pad_p)
        ag[:, 0, 0, SEQ:] = c2 * pad_p[:, SEQ - fw:]
        ins["ag"] = ag.astype(fp8)
    return ins, perms, fw, fused_fix


def kernel(zipped_top, zipped_left, indicator, padding_dist):
    global LAST_RESULTS
    from concourse.bass_utils import run_bass_kernel_spmd

    p = float(np.asarray(padding_dist))
    ins, perms, fw, fused_fix = _host_prep(
        zipped_top, zipped_left, indicator, p)

    key = (fw, fused_fix)
    if key not in _NC_CACHE:
        _NC_CACHE[key] = _build_nc(fw, fused_fix)
    nc = _NC_CACHE[key]

    in_maps = [{k: v[c] for k, v in ins.items()} for c in range(N_CORES)]
    res = run_bass_kernel_spmd(
        nc, in_maps, core_ids=list(range(N_CORES)),
        trace=os.environ.get("BASS_TRACE", "") == "1",
    )
    LAST_RESULTS = res
    full = np.stack([np.asarray(res.results[c]["out"], dtype=np.float32)
                     for c in range(N_CORES)])
    # mirror the skipped below-diagonal blocks (128-row granularity)
    for mb in range(1, MB):
        lo = mb * 128
        r = slice(lo, lo + 128)
        full[:, r, :lo] = full[:, :lo, r].transpose(0, 2, 1)
    # undo the pads-last permutation (rows and cols)
    inv = np.argsort(perms, axis=1)
    full = np.take_along_axis(full, inv[:, :, None], axis=1)
    full = np.take_along_axis(full, inv[:, None, :], axis=2)
    return full
